# revision 3
# baseline (speedup 1.0000x reference)
"""Bass/Tile kernel for nn_Head (softmax-first attention with post-softmax
strict-upper causal mask), SPMD over 8 TRN2 NeuronCores.

  q = x @ Wq; k = y @ Wk; v = y @ Wv        (B=4, N=M=4096, C=1024, D=128)
  a = softmax(q k^T / sqrt(D))              (full-row softmax)
  a = triu(a, k=1)                          (post-softmax mask, keeps j > i)
  out = a @ v

Sharding: core (b, h) = (core//2, core%2) handles batch b and the j-chunks
(128 cols each) of parity h (column split of K/V).  Each core produces a
partial numerator num^T[d, i] = sum_{own j>i} E[i,j] v[j,:] and a partial
denominator z[i] = sum_{own j} E[i,j]; the host combines
out = ((num0+num1)/(z0+z1))^T.

Scheme: scores are computed directly transposed, S^T[j, i] (kT chunk
stationary / qT streaming 512-wide), exp'd by ScalarE into E^T (bf16), the
denominator comes from DVE-folding E^T chunks 16->4 then ones-stationary
matmuls (fold-free direct matmuls for the last igroup so the tail stays
short), and AV streams E^T against a v-chunk stationary operand producing
out^T -- no PE transposes of attention tiles at all.  Emission is
slot-based so QK pairs, Z, AV and qT-projection matmuls interleave and the
PE never sits behind the (serial, ~73us) ScalarE exp stream.
"""
import sys
sys.path.insert(0, '/opt/trn_rl_repo')

from contextlib import ExitStack

import numpy as np
import ml_dtypes

import concourse.bass as bass
import concourse.bacc as bacc
import concourse.tile as tile
from concourse import mybir
from concourse.bass_utils import run_bass_kernel_spmd
from concourse.masks import make_identity

F32 = mybir.dt.float32
BF16 = mybir.dt.bfloat16
NPBF16 = ml_dtypes.bfloat16

B, N, M, C, D = 4, 4096, 4096, 1024, 128
NCORES = 8
MLOC = M // 2              # 2048 own j columns per core
LCH = MLOC // 128          # 16 own j-chunks
CCH = C // 128             # 8 contraction chunks
NG = N // 512              # 8 i-groups of 512 rows
SCALE = 1.0 / np.sqrt(np.float32(D))

_CACHE = {}
TRACE = False


def build_nc_spmd():
    nc = bacc.Bacc("TRN2", target_bir_lowering=False, debug=False,
                   num_devices=NCORES)
    xT_d = nc.dram_tensor("xT", [128, NG * CCH * 512], BF16,
                          kind="ExternalInput").ap()
    yT_d = nc.dram_tensor("yT", [128, 4 * CCH * 512], BF16,
                          kind="ExternalInput").ap()
    # packed constants: wk | wq | wv | bmask, one DMA
    cst_d = nc.dram_tensor("cst", [128, 4 * CCH * D], BF16,
                           kind="ExternalInput").ap()
    outT_d = nc.dram_tensor("outT", [D, N], F32, kind="ExternalOutput").ap()
    z_d = nc.dram_tensor("z", [1, N], F32, kind="ExternalOutput").ap()

    xT_view = xT_d.rearrange("p (t c j) -> p t c j", c=CCH, j=512)
    yT_view = yT_d.rearrange("p (t c j) -> p t c j", c=CCH, j=512)

    with tile.TileContext(nc) as tc:
        with ExitStack() as ctx:
            const = ctx.enter_context(tc.tile_pool(name="const", bufs=1))
            xstage = ctx.enter_context(tc.tile_pool(name="xstage", bufs=3))
            big = ctx.enter_context(tc.tile_pool(name="big", bufs=1))
            etp = ctx.enter_context(tc.tile_pool(name="etp", bufs=3))
            etm = ctx.enter_context(tc.tile_pool(name="etm", bufs=2))
            zf8p = ctx.enter_context(tc.tile_pool(name="zf8p", bufs=2))
            zf4p = ctx.enter_context(tc.tile_pool(name="zf4p", bufs=2))
            osb = ctx.enter_context(tc.tile_pool(name="osb", bufs=2))
            # PSUM: ST 3 bufs x 2 banks + Z 1 + AV 1 = 8 banks
            ps_st = ctx.enter_context(tc.tile_pool(name="ps_st", bufs=3,
                                                   space="PSUM"))
            ps_z = ctx.enter_context(tc.tile_pool(name="ps_z", bufs=1,
                                                  space="PSUM"))
            ps_av = ctx.enter_context(tc.tile_pool(name="ps_av", bufs=1,
                                                   space="PSUM"))

            # ---- constants in ONE DMA, then x tile 0, then yT tiles ----
            cst_sb = const.tile([128, 4, CCH, D], BF16)
            nc.sync.dma_start(out=cst_sb,
                              in_=cst_d.rearrange("p (w c d) -> p w c d",
                                                  c=CCH, d=D))
            wk_sb = cst_sb[:, 0]
            wq_sb = cst_sb[:, 1]
            wv_sb = cst_sb[:, 2]
            bmask_sb = cst_sb[:, 3].rearrange("p c d -> p (c d)").rearrange(
                "p (a b) -> p a b", b=512)
            xts = {}

            def qT_dma(it):
                xts[it] = xstage.tile([128, CCH, 512], BF16, tag="xt",
                                      name="xt")
                nc.sync.dma_start(out=xts[it], in_=xT_view[:, it])

            qT_dma(0)
            yT_sb = big.tile([128, 4, CCH, 512], BF16)
            for jt in range(4):
                nc.sync.dma_start(out=yT_sb[:, jt], in_=yT_view[:, jt])
            ident = const.tile([128, 128], BF16)
            make_identity(nc, ident)
            ones = const.tile([128, 1], BF16)
            nc.vector.memset(ones, 1.0)

            # ---- resident tensors ----
            kT_sb = big.tile([128, MLOC], BF16)        # [d, own j]
            vT_sb = big.tile([128, MLOC], BF16)        # [d, own j] staging
            v_sb = big.tile([128, LCH, D], BF16)       # [j-in-chunk, l, d]
            qT_sb = big.tile([128, N], BF16)           # [d, i]
            z_sb = big.tile([1, NG, 512], F32)         # denominators

            def kT_chain(jt):
                kp = ps_st.tile([128, 2, 512], F32, tag="st", name="kp")
                for c in range(CCH):
                    nc.tensor.matmul(kp[:, 0, :], wk_sb[:, c, :],
                                     yT_sb[:, jt, c, :],
                                     start=(c == 0), stop=(c == CCH - 1))
                nc.vector.tensor_copy(kT_sb[:, jt * 512:(jt + 1) * 512],
                                      kp[:, 0, :])

            def qT_mms(it):
                qp = ps_st.tile([128, 2, 512], F32, tag="st", name="qp")
                for c in range(CCH):
                    nc.tensor.matmul(qp[:, 0, :], wq_sb[:, c, :],
                                     xts[it][:, c, :],
                                     start=(c == 0), stop=(c == CCH - 1))
                nc.vector.tensor_copy(qT_sb[:, it * 512:(it + 1) * 512],
                                      qp[:, 0, :])

            qT_dma(1)
            qT_dma(2)
            qT_mms(0)
            kT_chain(0)

            # ---- state for the slot machine ----
            et_tiles = {}
            etm_tiles = {}
            zf4 = {}
            zps = ps_z.tile([128, 512], F32, tag="z", name="zps")
            z7st = {"ps": None, "done": 0}
            av_state = {}
            av_backlog = []            # (ready_slot, G, l)
            av_done = {G: 0 for G in range(NG)}
            vt_jobs = [(jt, c) for jt in range(4) for c in range(CCH)]
            vtps = {}

            def emit_qk_pair(G, p):
                if G not in et_tiles:
                    et_tiles[G] = etp.tile([128, LCH, 512], BF16, tag="et",
                                           name="et")
                et = et_tiles[G]
                st = ps_st.tile([128, 2, 512], F32, tag="st", name="st")
                for k in range(2):
                    l = 2 * p + k
                    nc.tensor.matmul(st[:, k, :],
                                     kT_sb[:, l * 128:(l + 1) * 128],
                                     qT_sb[:, G * 512:(G + 1) * 512],
                                     start=True, stop=True)
                nc.scalar.activation(
                    et[:, 2 * p:2 * p + 2, :].rearrange("p a b -> p (a b)"),
                    st.rearrange("p a b -> p (a b)"),
                    mybir.ActivationFunctionType.Exp,
                    scale=float(SCALE))

            def emit_mask(G):
                """Masked copies of the two band chunks into etm[G]."""
                et = et_tiles[G]
                em = etm.tile([128, 2, 512], BF16, tag="etm", name="em")
                etm_tiles[G] = em
                for i in range(2):
                    nc.vector.tensor_mul(em[:, i, :], et[:, 2 * G + i, :],
                                         bmask_sb[:, i, :])

            def emit_fold(G):
                et = et_tiles[G]
                ev = et.rearrange("p (a two) b -> p a two b", two=2)
                z8 = zf8p.tile([128, 8, 512], BF16, tag="z8", name="z8")
                nc.vector.tensor_add(z8, ev[:, :, 0, :], ev[:, :, 1, :])
                z8v = z8.rearrange("p (a two) b -> p a two b", two=2)
                z4 = zf4p.tile([128, 4, 512], BF16, tag="z4", name="z4")
                nc.vector.tensor_add(z4, z8v[:, :, 0, :], z8v[:, :, 1, :])
                zf4[G] = z4

            # igroup 7 pair emission order: band pair (7) first so its
            # mask/AV/out-DMA happen long before the tail
            P7 = [7, 0, 1, 2, 3, 4, 5, 6]

            def emit_z7(navail):
                """Fold-free denominator for the last igroup: direct
                ones-matmuls on E^T chunk pairs in emission order."""
                G = NG - 1
                while z7st["done"] < min(navail, 8):
                    e = z7st["done"]
                    if z7st["ps"] is None:
                        z7st["ps"] = ps_av.tile([128, 512], F32, tag="av",
                                                name="z7ps")
                    for k in range(2):
                        l = 2 * P7[e] + k
                        nc.tensor.matmul(z7st["ps"][0:1, :], ones,
                                         et_tiles[G][:, l, :],
                                         start=(e == 0 and k == 0),
                                         stop=(e == 7 and k == 1))
                    z7st["done"] = e + 1

            def queue_av(G, slot_now):
                """Queue AV matmuls for igroup G into the backlog."""
                base = 8 * G
                # band chunks (masked copies) first; ready after mask
                band_p = 0 if G == NG - 1 else G
                for i in range(2):
                    av_backlog.append((max(base + band_p + 3, 9), G,
                                       2 * G + i, ('em', i)))
                for l in range(2 * G + 2, LCH):
                    av_backlog.append((max(base + l // 2 + 3, 9), G,
                                       ('et', l)[1], ('et', l)))

            def emit_av_mm(G, l, src):
                if G not in av_state:
                    av_state[G] = ps_av.tile([128, 512], F32, tag="av",
                                             name="av")
                av = av_state[G]
                kind, idx = src
                rhs = (etm_tiles[G][:, idx, :] if kind == 'em'
                       else et_tiles[G][:, idx, :])
                nav = 16 - 2 * G
                av_done[G] += 1
                nc.tensor.matmul(av, v_sb[:, l, :], rhs,
                                 start=(av_done[G] == 1),
                                 stop=(av_done[G] == nav))
                if av_done[G] == nav:
                    ot = osb.tile([128, 512], F32, tag="ot", name="ot")
                    nc.vector.tensor_copy(ot, av)
                    nc.sync.dma_start(out=outT_d[:, G * 512:(G + 1) * 512],
                                      in_=ot)

            def drain_av(slot, budget):
                n = 0
                while n < budget and av_backlog and av_backlog[0][0] <= slot:
                    _, G, l, src = av_backlog.pop(0)
                    emit_av_mm(G, l, src)
                    n += 1

            # ---- slot machine: 64 pair-slots ----
            for t in range(8 * NG):
                G, p = divmod(t, 8)
                if p == 0:
                    queue_av(G, t)
                    if 3 <= G + 3 <= NG - 1:
                        qT_dma(G + 3)
                if G >= 1 and p == 1 and G + 2 <= NG - 1:
                    qT_mms(G + 2)
                if G == 1 and p == 3:
                    qT_mms(2)
                if G == 0 and p in (2, 4, 6):
                    kT_chain(p // 2)
                if G == 0 and p == 6:
                    qT_mms(1)
                pp = P7[p] if G == NG - 1 else p
                emit_qk_pair(G, pp)
                if G == 0:
                    # vT projection interleaved under igroup 0
                    if p % 2 == 0:
                        vtps[p // 2] = ps_st.tile([128, 2, 512], F32,
                                                  tag="st", name="vtps")
                    for jt, c in vt_jobs[p * 4:(p + 1) * 4]:
                        nc.tensor.matmul(vtps[jt][:, 1, :], wv_sb[:, c, :],
                                         yT_sb[:, jt, c, :],
                                         start=(c == 0), stop=(c == CCH - 1))
                    if p % 2 == 1:
                        jt = p // 2
                        nc.vector.tensor_copy(
                            vT_sb[:, jt * 512:(jt + 1) * 512],
                            vtps[jt][:, 1, :])
                    if p == 7:
                        # vT -> v transposes through PSUM
                        at = ps_st.tile([128, LCH, 128], BF16, tag="st",
                                        name="at")
                        for l in range(LCH):
                            nc.tensor.transpose(
                                at[:, l, :], vT_sb[:, l * 128:(l + 1) * 128],
                                ident)
                        nc.vector.tensor_copy(v_sb, at)
                if pp == G:
                    emit_mask(G)
                if p == 7 and G < NG - 1:
                    emit_fold(G)
                # Z matmuls for the previous igroup at slots 4..7
                if G >= 1 and 4 <= p <= 7:
                    nc.tensor.matmul(zps[0:1, :], ones, zf4[G - 1][:, p - 4, :],
                                     start=(p == 4), stop=(p == 7))
                    if p == 7:
                        nc.vector.tensor_copy(z_sb[0:1, G - 1, :], zps[0:1, :])
                drain_av(t, 3 if G > 0 else 2)
                if G == NG - 1 and p >= 3:
                    emit_z7(p - 2)

            # ---- tail ----
            emit_z7(8)
            nc.vector.tensor_copy(z_sb[0:1, NG - 1, :], z7st["ps"][0:1, :])
            drain_av(10 ** 9, 10 ** 9)
            nc.sync.dma_start(out=z_d, in_=z_sb.rearrange("p a b -> p (a b)"))

    nc.compile()
    return nc


def _get_nc():
    if "nc" not in _CACHE:
        _CACHE["nc"] = build_nc_spmd()
    return _CACHE["nc"]


def _make_bandmask(h):
    """Masks for the two band chunks of each igroup.

    For igroup G, band chunk l=2G+i (i in 0,1) is global chunk
    c = 2l + h = 4G + 2i + h.  Against the 4 blocks g = 4G + b (b=0..3):
      b <  bdiag: keep (1.0);  b == bdiag: strict j>i;  b > bdiag: zero
    with bdiag = 2i + h (independent of G).
    """
    m = np.zeros((128, 2, 512), dtype=np.float32)
    tri = np.tril(np.ones((128, 128), dtype=np.float32), k=-1)  # j > i
    for i in range(2):
        bdiag = 2 * i + h
        for b in range(4):
            if b < bdiag:
                m[:, i, b * 128:(b + 1) * 128] = 1.0
            elif b == bdiag:
                m[:, i, b * 128:(b + 1) * 128] = tri
    return np.ascontiguousarray(m.reshape(128, 1024)).astype(NPBF16)


def _shuffle_cp(a, ntile):
    """[rows=ntile*512, C] -> [128, ntile*CCH*512] with free index (t, c, j):
    a[t*512+j, c*128+p]."""
    r = a.reshape(ntile, 512, CCH, 128).transpose(3, 0, 2, 1)
    return np.ascontiguousarray(r.reshape(128, ntile * CCH * 512))


def kernel(x, y, Wq, Wk, Wv):
    nc = _get_nc()
    xb = np.asarray(x).astype(NPBF16)
    yb = np.asarray(y).astype(NPBF16)

    def wshuf(w):
        return np.ascontiguousarray(
            np.asarray(w).astype(NPBF16).reshape(CCH, 128, D)
            .transpose(1, 0, 2).reshape(128, CCH * D))

    wqb, wkb, wvb = wshuf(Wq), wshuf(Wk), wshuf(Wv)

    in_maps = []
    xTs = {b: _shuffle_cp(xb[b], NG) for b in range(B)}
    bmasks = {h: _make_bandmask(h) for h in range(2)}
    for core in range(NCORES):
        b, h = divmod(core, 2)
        yo = yb[b].reshape(M // 128, 128, C)[h::2].reshape(MLOC, C)
        in_maps.append({
            "xT": xTs[b],
            "yT": _shuffle_cp(yo, 4),
            "cst": np.ascontiguousarray(
                np.concatenate([wkb, wqb, wvb, bmasks[h]], axis=1)),
        })

    if TRACE:
        import tempfile
        tdir = tempfile.mkdtemp(prefix="attn_trace_")
        _CACHE["trace_dir"] = tdir
        res = run_bass_kernel_spmd(nc, in_maps, list(range(NCORES)),
                                   trace=True, tmpdir=tdir)
        _CACHE["exec_time_ns"] = res.exec_time_ns
    else:
        res = run_bass_kernel_spmd(nc, in_maps, list(range(NCORES)))

    out = np.empty((B, N, D), dtype=np.float32)
    for b in range(B):
        numT = res.results[2 * b]["outT"] + res.results[2 * b + 1]["outT"]
        z = (res.results[2 * b]["z"] + res.results[2 * b + 1]["z"]).reshape(N)
        out[b] = (numT / z[None, :]).T
    return out
